# revision 19
# baseline (speedup 1.0000x reference)
"""AttnBlock (GroupNorm + single-head 1x1-conv attention + residual) on 8
Trainium2 NeuronCores, data-parallel over the batch dimension (one image per
core, weights replicated).

Per-core dataflow (x: [512 ch, 1024 px]), v3 — fp8 DoubleRow + HAM warmup +
measured-cost engine balancing:

  DMA        : x as 8 half-tiles alternating the sync/scalar HWDGE rings so
               tiles land staggered at the GN-stats consumption rate; the
               fp8 weights ride behind x as two 512KB concats; smallcat on
               the gpsimd ring.
  warmup     : dummy bf16 matmuls keep the PE busy from t~=0.7us so the HAM
               clock gate reaches K=8/8 during the DMA/stats phase; bridge
               warmups cover the GN-finalize and h8 gaps (idle > ~1us
               re-throttles the PE to 1.2 GHz for ~4-8us windows).
  GN stats   : per-tile DVE reduce (sum) + ACT Square accum (sum sq) as
               tiles land -> group sums via indicator matmul -> rstd =
               Exp(-0.5*Ln(var+eps)) on ACT (Ln/Exp/Square/Identity in ONE
               table set -> single table load) -> broadcast matmul ->
               h8 = fp8(x*a+b): t0 DVE, t1 ACT, t2 gpsimd, t3 DVE.
  matmuls    : all big matmuls fp8e4 DoubleRow (K=256/matmul, measured
               216ns vs 427ns for the f32r pair it replaces):
               q8/k8 [c,hw] (q drains ACT w/ 1/sqrt(c)+bq folded, k drains
               DVE), vT8 [hw,c] (DVE + bias), S^T = k8^T q8 (exp on ACT,
               ib0 exps prioritized so U(ib0) never waits), denominator
               via fp8-ones DoubleRow matmuls interleaved in the S stream,
               1/den = Exp(-Ln(den)) on ACT -> bf16 ones outer-product
               broadcast, U = vT8 @ est8 with u8 = psu * rep at the DVE
               drain, proj fp8 + residual add on DVE, output streamed per
               (ib,ot) tile over three DMA rings.
"""

from contextlib import ExitStack

import numpy as np

import concourse.bass as bass
import concourse.tile as tile
from concourse import mybir
from concourse.bass_utils import run_bass_kernel_spmd
from concourse.vector_clock import ScopedClock

B, C, HH, WW = 8, 512, 32, 32
HW = HH * WW          # 1024 pixels
P = 128               # SBUF partitions
CT = C // P           # 4 channel tiles
TP = CT // 2          # 2 DoubleRow channel-tile pairs
JT = HW // P          # 8 pixel tiles (keys)
SP = JT // 2          # 4 DoubleRow pixel-tile pairs
NB = 512              # matmul moving free dim (one PSUM bank of fp32)
IB = HW // NB         # 2 query blocks
NG = 8                # groupnorm groups
GS = C // NG          # 64 channels per group
EPS = 1e-5
SCALE = float(1.0 / np.sqrt(np.float32(C)))
N_WARM = 33           # PE warmup matmuls before the GN group matmuls
N_BRIDGE1 = 5         # warmups bridging GN group sums -> broadcast
N_BRIDGE2 = 6         # warmups bridging GN broadcast -> q/k

F32 = mybir.dt.float32
MM_DT = mybir.dt.float32r
F8 = mybir.dt.float8e4
BF16 = mybir.dt.bfloat16
DR = mybir.MatmulPerfMode.DoubleRow
Identity = mybir.ActivationFunctionType.Identity
Square = mybir.ActivationFunctionType.Square
Exp = mybir.ActivationFunctionType.Exp
Ln = mybir.ActivationFunctionType.Ln


class _TC(tile.TileContext):
    """This container's walrus build rejects instructions carrying more than
    one sync-wait condition. After scheduling, hoist the extra waits of every
    multi-wait instruction into single-wait EventSemaphore instructions
    inserted just before it on the same engine (semantically identical)."""

    def _split_multiwait(self):
        nc = self.nc
        for bb in nc.main_func.blocks:
            insts = bb.instructions
            out = []
            changed = False
            for inst in insts:
                si = inst.sync_info
                if si is not None and si.on_wait and len(si.on_wait) > 1:
                    waits = list(si.on_wait)
                    si.on_wait = [waits[-1]]
                    for w in waits[:-1]:
                        wi = mybir.InstEventSemaphore(
                            name=nc.get_next_instruction_name()
                        )
                        wi.engine = inst.engine
                        wi.sync_info = mybir.SyncInfo(on_wait=[w], on_update=[])
                        out.append(wi)
                    changed = True
                out.append(inst)
            if changed:
                bb.instructions = out

    def _drain_and_barrier(self, tick_clock, wait_clock):
        nc = self.nc
        drain_inst = nc.sync.drain()
        wait_clock.add_sem_waits(
            drain_inst.ins, ScopedClock({None: tick_clock.global_clock})
        )
        self._split_multiwait()
        popped = nc._tile_sem_poison_stack.pop()
        assert popped is self._sem_poison


def _build():
    nc = bass.Bass()
    x = nc.dram_tensor("x", [C, HW], F32, kind="ExternalInput")
    # weight concats pre-transposed host-side to [p, j, t, o] so each
    # partition's 2KB rides one contiguous DMA descriptor
    wqk = nc.dram_tensor("wqk8", [P, 2 * CT * C], F8, kind="ExternalInput")
    wvp = nc.dram_tensor("wvp8", [P, 2 * CT * C], F8, kind="ExternalInput")
    bv = nc.dram_tensor("bv", [C], F32, kind="ExternalInput")
    smallcat = nc.dram_tensor(
        "smallcat", [P, 5 * CT + CT * NG + CT * P], F32, kind="ExternalInput"
    )
    out = nc.dram_tensor("out", [C, HW], F32, kind="ExternalOutput")

    with _TC(nc) as tc, ExitStack() as ctx:
        big = ctx.enter_context(tc.tile_pool(name="big", bufs=1))
        small = ctx.enter_context(tc.tile_pool(name="small", bufs=1))
        tmp = ctx.enter_context(tc.tile_pool(name="tmp", bufs=4))
        ps_pool = ctx.enter_context(tc.tile_pool(name="ps", bufs=5, space="PSUM"))
        ps_small = ctx.enter_context(tc.tile_pool(name="pss", bufs=2, space="PSUM"))
        ps_warm = ctx.enter_context(tc.tile_pool(name="psw", bufs=1, space="PSUM"))
        outp = ctx.enter_context(tc.tile_pool(name="outp", bufs=4))

        # ---- input DMAs ----
        # x as 8 half-tiles alternating two HWDGE rings -> tiles complete
        # staggered at ~the stats-engines' consumption rate; weight concats
        # ride behind x (landing just before first use); smallcat rides the
        # gpsimd (SWDGE) ring so it's there for the group matmuls.
        xsb = big.tile([P, CT, HW], F32, tag="xsb")
        xr = x.rearrange("(t p) i -> p t i", p=P)
        H2 = HW // 2
        # halves of tile t ride the sync HWDGE + gpsimd SWDGE rings (never
        # the scalar ring: a 3rd-in-flight DMA issue head-of-line blocks the
        # ACT compute queue). Tiles complete staggered at ~the stats rate.
        for t in range(CT):
            nc.sync.dma_start(out=xsb[:, t, 0:H2], in_=xr[:, t, 0:H2])
            nc.gpsimd.dma_start(out=xsb[:, t, H2:HW], in_=xr[:, t, H2:HW])
        w8 = big.tile([P, 2, 2, CT, C], F8, tag="w8")  # [qk/vp, 0/1, ct, o]
        # scalar ring gets exactly TWO issues (ring depth) -> no ACT
        # compute-queue blocking
        nc.scalar.dma_start(
            out=w8[:, 0], in_=wqk.rearrange("p (j t o) -> p j t o", j=2, t=CT)
        )
        nc.scalar.dma_start(
            out=w8[:, 1], in_=wvp.rearrange("p (j t o) -> p j t o", j=2, t=CT)
        )
        wq_sb = w8[:, 0, 0]
        wk_sb = w8[:, 0, 1]
        wv_sb = w8[:, 1, 0]
        wp_sb = w8[:, 1, 1]
        sc_sb = small.tile([P, 5 * CT + CT * NG + CT * P], F32, tag="smallcat")
        nc.gpsimd.dma_start(out=sc_sb[:], in_=smallcat[:])
        bv_sb = small.tile([P, C], F32, tag="bv")
        nc.gpsimd.dma_start(
            out=bv_sb[:],
            in_=bass.AP(tensor=bv[:].tensor, offset=0, ap=[[0, P], [1, C]]),
        )

        bq_sb = sc_sb[:, 0 * CT : 1 * CT]
        bk_sb = sc_sb[:, 1 * CT : 2 * CT]
        bp_sb = sc_sb[:, 2 * CT : 3 * CT]
        gs_sb = sc_sb[:, 3 * CT : 4 * CT]
        gb_sb = sc_sb[:, 4 * CT : 5 * CT]
        gmat_sb = sc_sb[:, 5 * CT : 5 * CT + CT * NG].rearrange(
            "p (t g) -> p t g", t=CT
        )
        hmat_sb = sc_sb[:, 5 * CT + CT * NG :].rearrange("p (t q) -> p t q", t=CT)

        # ---- PE warmup + small constants ----
        warm = small.tile([P, NB], BF16, tag="warm")
        nc.vector.memset(warm[:], 0.0)
        eps_sb = small.tile([P, 1], F32, tag="eps")
        nc.vector.memset(eps_sb[:], EPS)
        ones8 = small.tile([P, 2, 16], F8, tag="ones8")
        nc.vector.memset(ones8[:], 1.0)
        ones_row = small.tile([1, P], BF16, tag="onesr")
        nc.vector.memset(ones_row[:], 1.0)
        # trigger the single ACT table load (natural_log_exp set) early
        actwarm = tmp.tile([1, 1], F32, tag="actwarm")
        nc.scalar.activation(out=actwarm[:], in_=eps_sb[0:1, :], func=Exp)

        warm_ps = ps_warm.tile([P, NB], F32, tag="warmps")

        def warm_mms(n):
            for _ in range(n):
                nc.tensor.matmul(
                    warm_ps[:], warm[:, 0:P], warm[:], start=True, stop=True
                )

        warm_mms(N_WARM)

        # ---- groupnorm statistics (pipelined with the x DMA) ----
        mom = small.tile([P, CT, 2], F32, tag="mom")  # (sum, sum(x^2))
        sqscr = big.tile([P, HW], F32, tag="sqscr")  # dead Square output
        for t in range(CT):
            nc.vector.reduce_sum(
                out=mom[:, t, 0:1], in_=xsb[:, t, :], axis=mybir.AxisListType.X
            )
            nc.scalar.activation(
                out=sqscr[:],
                in_=xsb[:, t, :],
                func=Square,
                accum_out=mom[:, t, 1:2],
            )

        # group sums across partitions: [8, 2] = gmat.T @ mom
        ps_g = ps_small.tile([NG, 2], F32, tag="pssmall")
        for t in range(CT):
            nc.tensor.matmul(
                ps_g[:], gmat_sb[:, t, :], mom[:, t, :],
                start=(t == 0), stop=(t == CT - 1),
            )
        warm_mms(N_BRIDGE1)

        # finalize on 8 partitions: mu = S1/n, var = S2/n - mu^2,
        # rstd = exp(-0.5*ln(var+eps))
        gsf = small.tile([P, 2], F32, tag="gsf")
        nc.vector.memset(gsf[:], 0.0)
        sc = tmp.tile([P, 4], F32, tag="gnsc")
        nc.vector.tensor_scalar_mul(sc[0:NG, 0:1], ps_g[0:NG, 0:1], 1.0 / (GS * HW))
        nc.vector.tensor_scalar_mul(sc[0:NG, 1:2], ps_g[0:NG, 1:2], 1.0 / (GS * HW))
        nc.vector.tensor_tensor(
            out=sc[0:NG, 2:3], in0=sc[0:NG, 0:1], in1=sc[0:NG, 0:1],
            op=mybir.AluOpType.mult,
        )
        nc.vector.tensor_tensor(
            out=sc[0:NG, 3:4], in0=sc[0:NG, 1:2], in1=sc[0:NG, 2:3],
            op=mybir.AluOpType.subtract,
        )
        nc.vector.tensor_copy(out=gsf[0:NG, 0:1], in_=sc[0:NG, 0:1])
        lnv = tmp.tile([NG, 1], F32, tag="lnv")
        nc.scalar.activation(
            out=lnv[:], in_=sc[0:NG, 3:4], func=Ln, bias=eps_sb[0:NG, 0:1]
        )
        nc.scalar.activation(out=gsf[0:NG, 1:2], in_=lnv[:], func=Exp, scale=-0.5)

        # broadcast (mu, rstd) to channel partitions; fold affine:
        # a = rstd*gn_scale ; b = gn_bias - mu*a ; h8 = fp8(x*a + b)
        ab = small.tile([P, CT, 2], F32, tag="ab")
        for t in range(CT):
            ps_b = ps_pool.tile([P, 2], F32, tag="mmps", name="psb")
            nc.tensor.matmul(
                ps_b[:], hmat_sb[:, t, :], gsf[:], start=True, stop=True
            )
            nc.vector.tensor_tensor(
                out=ab[:, t, 0:1], in0=ps_b[:, 1:2], in1=gs_sb[:, t : t + 1],
                op=mybir.AluOpType.mult,
            )
            nc.vector.tensor_tensor(
                out=ab[:, t, 1:2], in0=ps_b[:, 0:1], in1=ab[:, t, 0:1],
                op=mybir.AluOpType.mult,
            )
            nc.vector.tensor_tensor(
                out=ab[:, t, 1:2], in0=gb_sb[:, t : t + 1], in1=ab[:, t, 1:2],
                op=mybir.AluOpType.subtract,
            )
        warm_mms(N_BRIDGE2)

        # h8 tiles: t0 DVE, t1 ACT, t2 gpsimd, t3 DVE -- the first pair
        # lands early so the tp0 q/k matmuls can start.
        h8 = big.tile([P, CT, HW], F8, tag="h8")
        h_engs = [nc.vector, nc.scalar, nc.gpsimd, nc.vector]
        for t in range(CT):
            e = h_engs[t]
            if e is nc.scalar:
                nc.scalar.activation(
                    out=h8[:, t, :], in_=xsb[:, t, :], func=Identity,
                    scale=ab[:, t, 0:1], bias=ab[:, t, 1:2],
                )
            else:
                e.tensor_scalar(
                    out=h8[:, t, :], in0=xsb[:, t, :],
                    scalar1=ab[:, t, 0:1], scalar2=ab[:, t, 1:2],
                    op0=mybir.AluOpType.mult, op1=mybir.AluOpType.add,
                )

        # ---- q / k projections, [c, hw] layout, fp8 DoubleRow ----
        # q drains on ACT (scale+bias folded), k drains on DVE.
        q8 = big.tile([P, CT, HW], F8, tag="q8")
        k8 = big.tile([P, CT, HW], F8, tag="k8")
        for ot in range(CT):
            osl = slice(ot * P, (ot + 1) * P)
            pss = {}
            for name in ("q", "k"):
                for ib in range(IB):
                    pss[name, ib] = ps_pool.tile(
                        [P, NB], F32, tag="mmps", name="psqk"
                    )
            for tp in range(TP):
                tsl = slice(2 * tp, 2 * tp + 2)
                for name, w_sb in (("q", wq_sb), ("k", wk_sb)):
                    for ib in range(IB):
                        isl = slice(ib * NB, (ib + 1) * NB)
                        nc.tensor.matmul(
                            pss[name, ib][:],
                            w_sb[:, tsl, osl],
                            h8[:, tsl, isl],
                            start=(tp == 0), stop=(tp == TP - 1),
                            perf_mode=DR,
                        )
            for ib in range(IB):
                isl = slice(ib * NB, (ib + 1) * NB)
                # q = (wq.h)*(1/sqrt(c)) + bq/sqrt(c); bq pre-scaled on host
                nc.scalar.activation(
                    out=q8[:, ot, isl], in_=pss["q", ib][:],
                    func=Identity, bias=bq_sb[:, ot : ot + 1], scale=SCALE,
                )
                nc.vector.tensor_scalar_add(
                    k8[:, ot, isl], pss["k", ib][:], bk_sb[:, ot : ot + 1]
                )

        # ---- vT projection, [hw, c] layout, fp8 DoubleRow, DVE drains ----
        vT8 = big.tile([P, JT, C], F8, tag="vT8")
        with nc.allow_low_precision(reason="fp8 attention path (validated)"):
            for jt in range(JT):
                jsl = slice(jt * P, (jt + 1) * P)
                psv = ps_pool.tile([P, NB], F32, tag="mmps", name="psv")
                for tp in range(TP):
                    tsl = slice(2 * tp, 2 * tp + 2)
                    nc.tensor.matmul(
                        psv[:],
                        h8[:, tsl, jsl],
                        wv_sb[:, tsl, :],
                        start=(tp == 0), stop=(tp == TP - 1),
                        perf_mode=DR,
                    )
                nc.vector.tensor_tensor(
                    out=vT8[:, jt, :], in0=psv[:], in1=bv_sb[:],
                    op=mybir.AluOpType.add,
                )

        # ---- S^T = k8^T q8 (DoubleRow); exp on ACT -> est8 fp8 ----
        # ib0 exps are emitted two jts ahead of ib1 exps so U(ib0) never
        # waits on ACT; denominator matmuls (fp8 ones) interleave per pair.
        est8 = big.tile([P, JT, HW], F8, tag="est8")
        ps_den = [
            ps_small.tile([1, NB], F32, tag="pssmall", name="psden")
            for _ in range(IB)
        ]
        ps_s = {}

        def s_exp(jt, ib):
            isl = slice(ib * NB, (ib + 1) * NB)
            nc.scalar.activation(
                out=est8[:, jt, isl], in_=ps_s.pop((jt, ib))[:], func=Exp
            )

        for jt in range(JT):
            jsl = slice(jt * P, (jt + 1) * P)
            for ib in range(IB):
                ps_s[jt, ib] = ps_pool.tile(
                    [P, NB], F32, tag="mmps", name="pss"
                )
            for tp in range(TP):
                tsl = slice(2 * tp, 2 * tp + 2)
                for ib in range(IB):
                    isl = slice(ib * NB, (ib + 1) * NB)
                    nc.tensor.matmul(
                        ps_s[jt, ib][:],
                        k8[:, tsl, jsl],
                        q8[:, tsl, isl],
                        start=(tp == 0), stop=(tp == TP - 1),
                        perf_mode=DR,
                    )
            s_exp(jt, 0)
            if jt >= 2:
                s_exp(jt - 2, 1)
            # den MMs: ib0 pairs as soon as both exps exist; ib1 two jts late
            if jt % 2 == 1:
                s = jt // 2
                ssl = slice(2 * s, 2 * s + 2)
                nc.tensor.matmul(
                    ps_den[0][:], ones8[:, :, 0:1], est8[:, ssl, 0:NB],
                    start=(s == 0), stop=(s == SP - 1),
                    perf_mode=DR, skip_group_check=True,
                )
                if s >= 1:
                    pssl = slice(2 * (s - 1), 2 * (s - 1) + 2)
                    nc.tensor.matmul(
                        ps_den[1][:], ones8[:, :, 0:1], est8[:, pssl, NB:HW],
                        start=(s == 1), stop=False,
                        perf_mode=DR, skip_group_check=True,
                    )
        s_exp(6, 1)
        s_exp(7, 1)
        nc.tensor.matmul(
            ps_den[1][:], ones8[:, :, 0:1], est8[:, 6:8, NB:HW],
            start=False, stop=True, perf_mode=DR, skip_group_check=True,
        )

        # ---- denominator -> rep = broadcast(1/den) ----
        # 1/den = Exp(-Ln(den)) on ACT; bf16 ones outer-product broadcast.
        lnden = tmp.tile([1, HW], F32, tag="lnden")
        recip_r = small.tile([1, HW], BF16, tag="recipr")
        rep = small.tile([P, HW], F32, tag="rep")

        def rep_chain_act(ib):
            isl = slice(ib * NB, (ib + 1) * NB)
            nc.scalar.activation(out=lnden[:, isl], in_=ps_den[ib][:], func=Ln)
            nc.scalar.activation(
                out=recip_r[:, isl], in_=lnden[:, isl], func=Exp, scale=-1.0
            )

        ps_rep = []

        def rep_mm(ib):
            isl = slice(ib * NB, (ib + 1) * NB)
            ps_r = ps_small.tile([P, NB], F32, tag="pssmall", name="psrep")
            nc.tensor.matmul(
                ps_r[:], ones_row[:], recip_r[:, isl], start=True, stop=True
            )
            ps_rep.append(ps_r)
            nc.vector.tensor_copy(
                out=rep[:, isl], in_=ps_r[:]
            )

        rep_chain_act(0)
        rep_chain_act(1)

        # ---- U = vT8 @ est8 (DoubleRow); u8 = psu * rep at the DVE drain --
        # rep matmuls slot in after the 2nd/6th U group so the PE never
        # stalls on the (ACT-backlogged) reciprocal chain; u8 drains are
        # deferred two groups so each is emitted AFTER its rep broadcast.
        u8 = big.tile([P, CT, HW], F8, tag="u8")
        pend = []

        def u_drain(ib, ct, psu):
            isl = slice(ib * NB, (ib + 1) * NB)
            nc.vector.tensor_tensor(
                out=u8[:, ct, isl], in0=psu[:], in1=rep[:, isl],
                op=mybir.AluOpType.mult,
            )

        for ib in range(IB):
            isl = slice(ib * NB, (ib + 1) * NB)
            for ct in range(CT):
                csl = slice(ct * P, (ct + 1) * P)
                psu = ps_pool.tile([P, NB], F32, tag="mmps", name="psu")
                for s in range(SP):
                    ssl = slice(2 * s, 2 * s + 2)
                    nc.tensor.matmul(
                        psu[:],
                        vT8[:, ssl, csl],
                        est8[:, ssl, isl],
                        start=(s == 0), stop=(s == SP - 1),
                        perf_mode=DR,
                    )
                if ib == 0 and ct == 1:
                    rep_mm(0)
                if ib == 1 and ct == 1:
                    rep_mm(1)
                pend.append((ib, ct, psu))
                while len(pend) > 2:
                    u_drain(*pend.pop(0))
        while pend:
            u_drain(*pend.pop(0))

        # ---- proj (fp8 DoubleRow) + residual on DVE -> staged output ----
        # drains land in a staging tile; each ot ships as ONE 512KB DMA on
        # its own ring as soon as both ib halves are drained (per-ring
        # transfers serialize, so one transfer per ring minimizes the tail).
        outr = out.rearrange("(t p) i -> p t i", p=P)
        ost = big.tile([P, CT, HW], F32, tag="ost")
        out_engs = [nc.sync, nc.gpsimd, nc.scalar, nc.sync]
        for ib in range(IB):
            isl = slice(ib * NB, (ib + 1) * NB)
            for ot in range(CT):
                osl = slice(ot * P, (ot + 1) * P)
                psp = ps_pool.tile([P, NB], F32, tag="mmps", name="psp")
                for tp in range(TP):
                    tsl = slice(2 * tp, 2 * tp + 2)
                    nc.tensor.matmul(
                        psp[:],
                        wp_sb[:, tsl, osl],
                        u8[:, tsl, isl],
                        start=(tp == 0), stop=(tp == TP - 1),
                        perf_mode=DR,
                    )
                # out = (psp + bproj) + x in one DVE pass
                nc.vector.scalar_tensor_tensor(
                    out=ost[:, ot, isl], in0=psp[:],
                    scalar=bp_sb[:, ot : ot + 1], in1=xsb[:, ot, isl],
                    op0=mybir.AluOpType.add, op1=mybir.AluOpType.add,
                )
                if ib == IB - 1:
                    out_engs[ot].dma_start(
                        out=outr[:, ot, :], in_=ost[:, ot, :]
                    )
    return nc


_NC = None


def _get_nc():
    global _NC
    if _NC is None:
        _NC = _build()
    return _NC


def _prep_inputs(x, gn_scale, gn_bias, wq, bq, wk, bk, wv, bv, wproj, bproj):
    import ml_dtypes

    f = np.float32
    f8 = ml_dtypes.float8_e4m3
    x = np.ascontiguousarray(x, dtype=f).reshape(B, C, HW)

    def t8(w):  # [o, c] -> [c, o] in fp8
        return np.asarray(w, dtype=f).T.astype(f8)

    def pt(v):  # [512] -> [128, 4] with v[t*128 + p] at [p, t]
        return np.ascontiguousarray(np.asarray(v, dtype=f).reshape(CT, P).T)

    pidx = np.arange(P)[:, None]
    tidx = np.arange(CT)[None, :]
    grp = 2 * tidx + pidx // GS  # [128, 4] group id per (p, t)
    gmat = np.zeros((P, CT, NG), f)
    hmat = np.zeros((P, CT, P), f)
    for t in range(CT):
        gmat[pidx[:, 0], t, grp[:, t]] = 1.0
        hmat[grp[:, t], t, pidx[:, 0]] = 1.0

    smallcat = np.concatenate(
        [
            pt(np.asarray(bq, dtype=f) * np.float32(SCALE)),
            pt(bk), pt(bproj), pt(gn_scale), pt(gn_bias),
            gmat.reshape(P, CT * NG), hmat.reshape(P, CT * P),
        ],
        axis=1,
    )
    def wcat(a, b):  # two [o,c] -> [p, (j t o)] fp8, per-partition contiguous
        w = np.stack([t8(a), t8(b)])  # [j, c, o]
        w = w.reshape(2, CT, P, C).transpose(2, 0, 1, 3)  # [p, j, t, o]
        return np.ascontiguousarray(w.reshape(P, 2 * CT * C))

    shared = {
        "wqk8": wcat(wq, wk),
        "wvp8": wcat(wv, wproj),
        "bv": np.ascontiguousarray(np.asarray(bv, dtype=f)),
        "smallcat": np.ascontiguousarray(smallcat),
    }
    return [dict(shared, x=np.ascontiguousarray(x[b])) for b in range(B)]


def _run(inputs, **kw):
    nc = _get_nc()
    in_maps = _prep_inputs(**inputs)
    return run_bass_kernel_spmd(nc, in_maps, core_ids=list(range(B)), **kw)


def kernel(**inputs) -> np.ndarray:
    res = _run(inputs)
    out = np.stack([res.results[b]["out"] for b in range(B)])
    return out.reshape(B, C, HH, WW).astype(np.float32)


# revision 21
# speedup vs baseline: 1.1035x; 1.1035x over previous
"""AttnBlock (GroupNorm + single-head 1x1-conv attention + residual) on 8
Trainium2 NeuronCores, data-parallel over the batch dimension (one image per
core, weights replicated).

Per-core dataflow (x: [512 ch, 1024 px]), v3 — fp8 DoubleRow + HAM warmup +
measured-cost engine balancing:

  DMA        : x as 8 half-tiles alternating the sync/scalar HWDGE rings so
               tiles land staggered at the GN-stats consumption rate; the
               fp8 weights ride behind x as two 512KB concats; smallcat on
               the gpsimd ring.
  warmup     : dummy bf16 matmuls keep the PE busy from t~=0.7us so the HAM
               clock gate reaches K=8/8 during the DMA/stats phase; bridge
               warmups cover the GN-finalize and h8 gaps (idle > ~1us
               re-throttles the PE to 1.2 GHz for ~4-8us windows).
  GN stats   : per-tile DVE reduce (sum) + ACT Square accum (sum sq) as
               tiles land -> group sums via indicator matmul -> rstd =
               Exp(-0.5*Ln(var+eps)) on ACT (Ln/Exp/Square/Identity in ONE
               table set -> single table load) -> broadcast matmul ->
               h8 = fp8(x*a+b): t0 DVE, t1 ACT, t2 gpsimd, t3 DVE.
  matmuls    : all big matmuls fp8e4 DoubleRow (K=256/matmul, measured
               216ns vs 427ns for the f32r pair it replaces):
               q8/k8 [c,hw] (q drains ACT w/ 1/sqrt(c)+bq folded, k drains
               DVE), vT8 [hw,c] (DVE + bias), S^T = k8^T q8 (exp on ACT,
               ib0 exps prioritized so U(ib0) never waits), denominator
               via fp8-ones DoubleRow matmuls interleaved in the S stream,
               1/den = Exp(-Ln(den)) on ACT -> bf16 ones outer-product
               broadcast, U = vT8 @ est8 with u8 = psu * rep at the DVE
               drain, proj fp8 + residual add on DVE, output streamed per
               (ib,ot) tile over three DMA rings.
"""

from contextlib import ExitStack

import numpy as np

import concourse.bass as bass
import concourse.tile as tile
from concourse import mybir
from concourse.bass_utils import run_bass_kernel_spmd
from concourse.vector_clock import ScopedClock

B, C, HH, WW = 8, 512, 32, 32
HW = HH * WW          # 1024 pixels
P = 128               # SBUF partitions
CT = C // P           # 4 channel tiles
TP = CT // 2          # 2 DoubleRow channel-tile pairs
JT = HW // P          # 8 pixel tiles (keys)
SP = JT // 2          # 4 DoubleRow pixel-tile pairs
NB = 512              # matmul moving free dim (one PSUM bank of fp32)
IB = HW // NB         # 2 query blocks
NG = 8                # groupnorm groups
GS = C // NG          # 64 channels per group
EPS = 1e-5
SCALE = float(1.0 / np.sqrt(np.float32(C)))
N_WARM = 33           # PE warmup matmuls before the GN group matmuls
N_BRIDGE1 = 5         # warmups bridging GN group sums -> broadcast
N_BRIDGE2 = 6         # warmups bridging GN broadcast -> q/k

F32 = mybir.dt.float32
MM_DT = mybir.dt.float32r
F8 = mybir.dt.float8e4
BF16 = mybir.dt.bfloat16
DR = mybir.MatmulPerfMode.DoubleRow
Identity = mybir.ActivationFunctionType.Identity
Square = mybir.ActivationFunctionType.Square
Exp = mybir.ActivationFunctionType.Exp
Ln = mybir.ActivationFunctionType.Ln


class _TC(tile.TileContext):
    """This container's walrus build rejects instructions carrying more than
    one sync-wait condition. After scheduling, hoist the extra waits of every
    multi-wait instruction into single-wait EventSemaphore instructions
    inserted just before it on the same engine (semantically identical)."""

    def _split_multiwait(self):
        nc = self.nc
        for bb in nc.main_func.blocks:
            insts = bb.instructions
            out = []
            changed = False
            for inst in insts:
                si = inst.sync_info
                if si is not None and si.on_wait and len(si.on_wait) > 1:
                    waits = list(si.on_wait)
                    si.on_wait = [waits[-1]]
                    for w in waits[:-1]:
                        wi = mybir.InstEventSemaphore(
                            name=nc.get_next_instruction_name()
                        )
                        wi.engine = inst.engine
                        wi.sync_info = mybir.SyncInfo(on_wait=[w], on_update=[])
                        out.append(wi)
                    changed = True
                out.append(inst)
            if changed:
                bb.instructions = out

    def _drain_and_barrier(self, tick_clock, wait_clock):
        nc = self.nc
        drain_inst = nc.sync.drain()
        wait_clock.add_sem_waits(
            drain_inst.ins, ScopedClock({None: tick_clock.global_clock})
        )
        self._split_multiwait()
        popped = nc._tile_sem_poison_stack.pop()
        assert popped is self._sem_poison


def _build():
    nc = bass.Bass()
    x = nc.dram_tensor("x", [C, HW], MM_DT, kind="ExternalInput")
    # weight concat pre-transposed host-side to [p, (j t o)] so each
    # partition's 4KB rides one contiguous DMA descriptor; identity matrix
    # (f32r) folds the residual add into the proj matmul group.
    wcat = nc.dram_tensor("wcat8", [P, 4 * CT * C], F8, kind="ExternalInput")
    ident = nc.dram_tensor("ident", [P, P], MM_DT, kind="ExternalInput")
    bv = nc.dram_tensor("bv", [C], F32, kind="ExternalInput")
    smallcat = nc.dram_tensor(
        "smallcat", [P, 5 * CT + CT * NG + CT * P], F32, kind="ExternalInput"
    )
    out = nc.dram_tensor("out", [C, HW], F32, kind="ExternalOutput")

    with _TC(nc) as tc, ExitStack() as ctx:
        big = ctx.enter_context(tc.tile_pool(name="big", bufs=1))
        small = ctx.enter_context(tc.tile_pool(name="small", bufs=1))
        tmp = ctx.enter_context(tc.tile_pool(name="tmp", bufs=4))
        ps_pool = ctx.enter_context(tc.tile_pool(name="ps", bufs=5, space="PSUM"))
        ps_small = ctx.enter_context(tc.tile_pool(name="pss", bufs=2, space="PSUM"))
        ps_warm = ctx.enter_context(tc.tile_pool(name="psw", bufs=1, space="PSUM"))
        outp = ctx.enter_context(tc.tile_pool(name="outp", bufs=4))

        # ---- input DMAs ----
        # x as 8 half-tiles alternating two HWDGE rings -> tiles complete
        # staggered at ~the stats-engines' consumption rate; weight concats
        # ride behind x (landing just before first use); smallcat rides the
        # gpsimd (SWDGE) ring so it's there for the group matmuls.
        # x tiles: t0/t3 on the sync ring, t1 on scalar, t2 on gpsimd --
        # max ring parallelism with at most 2 transfers per HWDGE ring (a
        # 3rd in-flight issue head-of-line blocks the issuing engine's
        # compute queue); each transfer is a full 512KB tile to amortize
        # the ~2-3us per-transfer completion latency.
        xsb = big.tile([P, CT, HW], MM_DT, tag="xsb")
        xr = x.rearrange("(t p) i -> p t i", p=P)
        nc.sync.dma_start(out=xsb[:, 0, :], in_=xr[:, 0, :])
        nc.scalar.dma_start(out=xsb[:, 1, :], in_=xr[:, 1, :])
        sc_sb = small.tile([P, 5 * CT + CT * NG + CT * P], F32, tag="smallcat")
        nc.gpsimd.dma_start(out=sc_sb[:], in_=smallcat[:])
        bv_sb = small.tile([P, C], F32, tag="bv")
        nc.gpsimd.dma_start(
            out=bv_sb[:],
            in_=bass.AP(tensor=bv[:].tensor, offset=0, ap=[[0, P], [1, C]]),
        )
        id_sb = small.tile([P, P], MM_DT, tag="ident")
        nc.gpsimd.dma_start(out=id_sb[:], in_=ident[:])
        nc.gpsimd.dma_start(out=xsb[:, 2, :], in_=xr[:, 2, :])
        nc.sync.dma_start(out=xsb[:, 3, :], in_=xr[:, 3, :])
        w8 = big.tile([P, 4, CT, C], F8, tag="w8")  # [q/k/v/p, ct, o]
        nc.scalar.dma_start(
            out=w8[:], in_=wcat.rearrange("p (j t o) -> p j t o", j=4, t=CT)
        )
        wq_sb = w8[:, 0]
        wk_sb = w8[:, 1]
        wv_sb = w8[:, 2]
        wp_sb = w8[:, 3]

        bq_sb = sc_sb[:, 0 * CT : 1 * CT]
        bk_sb = sc_sb[:, 1 * CT : 2 * CT]
        bp_sb = sc_sb[:, 2 * CT : 3 * CT]
        gs_sb = sc_sb[:, 3 * CT : 4 * CT]
        gb_sb = sc_sb[:, 4 * CT : 5 * CT]
        gmat_sb = sc_sb[:, 5 * CT : 5 * CT + CT * NG].rearrange(
            "p (t g) -> p t g", t=CT
        )
        hmat_sb = sc_sb[:, 5 * CT + CT * NG :].rearrange("p (t q) -> p t q", t=CT)

        # ---- PE warmup + small constants ----
        warm = small.tile([P, NB], BF16, tag="warm")
        nc.vector.memset(warm[:], 0.0)
        eps_sb = small.tile([P, 1], F32, tag="eps")
        nc.vector.memset(eps_sb[:], EPS)
        ones8 = small.tile([P, 2, 16], F8, tag="ones8")
        nc.vector.memset(ones8[:], 1.0)
        ones_row = small.tile([1, P], BF16, tag="onesr")
        nc.vector.memset(ones_row[:], 1.0)
        # trigger the single ACT table load (natural_log_exp set) early
        actwarm = tmp.tile([1, 1], F32, tag="actwarm")
        nc.scalar.activation(out=actwarm[:], in_=eps_sb[0:1, :], func=Exp)

        warm_ps = ps_warm.tile([P, NB], F32, tag="warmps")

        def warm_mms(n):
            for _ in range(n):
                nc.tensor.matmul(
                    warm_ps[:], warm[:, 0:P], warm[:], start=True, stop=True
                )

        warm_mms(N_WARM)

        # ---- groupnorm statistics (pipelined with the x DMA) ----
        mom = small.tile([P, CT, 2], F32, tag="mom")  # (sum, sum(x^2))
        sqscr = big.tile([P, HW], F32, tag="sqscr")  # dead Square output
        for t in range(CT):
            nc.vector.reduce_sum(
                out=mom[:, t, 0:1], in_=xsb[:, t, :], axis=mybir.AxisListType.X
            )
            nc.scalar.activation(
                out=sqscr[:],
                in_=xsb[:, t, :],
                func=Square,
                accum_out=mom[:, t, 1:2],
            )

        # group sums across partitions: [8, 2] = gmat.T @ mom
        ps_g = ps_small.tile([NG, 2], F32, tag="pssmall")
        for t in range(CT):
            nc.tensor.matmul(
                ps_g[:], gmat_sb[:, t, :], mom[:, t, :],
                start=(t == 0), stop=(t == CT - 1),
            )
        warm_mms(N_BRIDGE1)

        # finalize on 8 partitions: mu = S1/n, var = S2/n - mu^2,
        # rstd = exp(-0.5*ln(var+eps))
        gsf = small.tile([P, 2], F32, tag="gsf")
        nc.vector.memset(gsf[:], 0.0)
        sc = tmp.tile([P, 4], F32, tag="gnsc")
        nc.vector.tensor_scalar_mul(sc[0:NG, 0:1], ps_g[0:NG, 0:1], 1.0 / (GS * HW))
        nc.vector.tensor_scalar_mul(sc[0:NG, 1:2], ps_g[0:NG, 1:2], 1.0 / (GS * HW))
        nc.vector.tensor_tensor(
            out=sc[0:NG, 2:3], in0=sc[0:NG, 0:1], in1=sc[0:NG, 0:1],
            op=mybir.AluOpType.mult,
        )
        nc.vector.tensor_tensor(
            out=sc[0:NG, 3:4], in0=sc[0:NG, 1:2], in1=sc[0:NG, 2:3],
            op=mybir.AluOpType.subtract,
        )
        nc.vector.tensor_copy(out=gsf[0:NG, 0:1], in_=sc[0:NG, 0:1])
        lnv = tmp.tile([NG, 1], F32, tag="lnv")
        nc.scalar.activation(
            out=lnv[:], in_=sc[0:NG, 3:4], func=Ln, bias=eps_sb[0:NG, 0:1]
        )
        nc.scalar.activation(out=gsf[0:NG, 1:2], in_=lnv[:], func=Exp, scale=-0.5)

        # broadcast (mu, rstd) to channel partitions; fold affine:
        # a = rstd*gn_scale ; b = gn_bias - mu*a ; h8 = fp8(x*a + b)
        ab = small.tile([P, CT, 2], F32, tag="ab")
        for t in range(CT):
            ps_b = ps_pool.tile([P, 2], F32, tag="mmps", name="psb")
            nc.tensor.matmul(
                ps_b[:], hmat_sb[:, t, :], gsf[:], start=True, stop=True
            )
            nc.vector.tensor_tensor(
                out=ab[:, t, 0:1], in0=ps_b[:, 1:2], in1=gs_sb[:, t : t + 1],
                op=mybir.AluOpType.mult,
            )
            nc.vector.tensor_tensor(
                out=ab[:, t, 1:2], in0=ps_b[:, 0:1], in1=ab[:, t, 0:1],
                op=mybir.AluOpType.mult,
            )
            nc.vector.tensor_tensor(
                out=ab[:, t, 1:2], in0=gb_sb[:, t : t + 1], in1=ab[:, t, 1:2],
                op=mybir.AluOpType.subtract,
            )
        warm_mms(N_BRIDGE2)

        # h8 tiles: t0 DVE, t1 ACT, t2 gpsimd, t3 DVE -- the first pair
        # lands early so the tp0 q/k matmuls can start.
        h8 = big.tile([P, CT, HW], F8, tag="h8")
        h_engs = [nc.vector, nc.scalar, nc.gpsimd, nc.vector]
        for t in range(CT):
            e = h_engs[t]
            if e is nc.scalar:
                nc.scalar.activation(
                    out=h8[:, t, :], in_=xsb[:, t, :], func=Identity,
                    scale=ab[:, t, 0:1], bias=ab[:, t, 1:2],
                )
            else:
                e.tensor_scalar(
                    out=h8[:, t, :], in0=xsb[:, t, :],
                    scalar1=ab[:, t, 0:1], scalar2=ab[:, t, 1:2],
                    op0=mybir.AluOpType.mult, op1=mybir.AluOpType.add,
                )

        # ---- q / k projections, [c, hw] layout, fp8 DoubleRow ----
        # q drains on ACT (scale+bias folded), k drains on DVE.
        q8 = big.tile([P, CT, HW], F8, tag="q8")
        k8 = big.tile([P, CT, HW], F8, tag="k8")
        for ot in range(CT):
            osl = slice(ot * P, (ot + 1) * P)
            pss = {}
            for name in ("q", "k"):
                for ib in range(IB):
                    pss[name, ib] = ps_pool.tile(
                        [P, NB], F32, tag="mmps", name="psqk"
                    )
            for tp in range(TP):
                tsl = slice(2 * tp, 2 * tp + 2)
                for name, w_sb in (("q", wq_sb), ("k", wk_sb)):
                    for ib in range(IB):
                        isl = slice(ib * NB, (ib + 1) * NB)
                        nc.tensor.matmul(
                            pss[name, ib][:],
                            w_sb[:, tsl, osl],
                            h8[:, tsl, isl],
                            start=(tp == 0), stop=(tp == TP - 1),
                            perf_mode=DR,
                        )
            for ib in range(IB):
                isl = slice(ib * NB, (ib + 1) * NB)
                # q = (wq.h)*(1/sqrt(c)) + bq/sqrt(c); bq pre-scaled on host
                nc.scalar.activation(
                    out=q8[:, ot, isl], in_=pss["q", ib][:],
                    func=Identity, bias=bq_sb[:, ot : ot + 1], scale=SCALE,
                )
                nc.vector.tensor_scalar_add(
                    k8[:, ot, isl], pss["k", ib][:], bk_sb[:, ot : ot + 1]
                )

        # ---- vT projection, [hw, c] layout, fp8 DoubleRow, DVE drains ----
        vT8 = big.tile([P, JT, C], F8, tag="vT8")
        with nc.allow_low_precision(reason="fp8 attention path (validated)"):
            for jt in range(JT):
                jsl = slice(jt * P, (jt + 1) * P)
                psv = ps_pool.tile([P, NB], F32, tag="mmps", name="psv")
                for tp in range(TP):
                    tsl = slice(2 * tp, 2 * tp + 2)
                    nc.tensor.matmul(
                        psv[:],
                        h8[:, tsl, jsl],
                        wv_sb[:, tsl, :],
                        start=(tp == 0), stop=(tp == TP - 1),
                        perf_mode=DR,
                    )
                nc.vector.tensor_tensor(
                    out=vT8[:, jt, :], in0=psv[:], in1=bv_sb[:],
                    op=mybir.AluOpType.add,
                )

        # ---- S^T = k8^T q8 (DoubleRow); exp on ACT -> est8 fp8 ----
        # ib0 exps are emitted two jts ahead of ib1 exps so U(ib0) never
        # waits on ACT; denominator matmuls (fp8 ones) interleave per pair.
        est8 = big.tile([P, JT, HW], F8, tag="est8")
        lnden = tmp.tile([1, HW], F32, tag="lnden")
        recip_r = small.tile([1, HW], BF16, tag="recipr")
        rep = small.tile([P, HW], F32, tag="rep")
        ps_rep = []

        ps_den = [
            ps_small.tile([1, NB], F32, tag="pssmall", name="psden")
            for _ in range(IB)
        ]
        ps_s = {}

        def rep_chain_act(ib):
            isl = slice(ib * NB, (ib + 1) * NB)
            nc.scalar.activation(out=lnden[:, isl], in_=ps_den[ib][:], func=Ln)
            nc.scalar.activation(
                out=recip_r[:, isl], in_=lnden[:, isl], func=Exp, scale=-1.0
            )

        def rep_mm(ib):
            isl = slice(ib * NB, (ib + 1) * NB)
            ps_r = ps_small.tile([P, NB], F32, tag="pssmall", name="psrep")
            nc.tensor.matmul(
                ps_r[:], ones_row[:], recip_r[:, isl], start=True, stop=True
            )
            ps_rep.append(ps_r)
            nc.vector.tensor_copy(
                out=rep[:, isl], in_=ps_r[:]
            )

        def s_exp(jt, ib):
            isl = slice(ib * NB, (ib + 1) * NB)
            nc.scalar.activation(
                out=est8[:, jt, isl], in_=ps_s.pop((jt, ib))[:], func=Exp
            )

        for jt in range(JT):
            jsl = slice(jt * P, (jt + 1) * P)
            for ib in range(IB):
                ps_s[jt, ib] = ps_pool.tile(
                    [P, NB], F32, tag="mmps", name="pss"
                )
            for tp in range(TP):
                tsl = slice(2 * tp, 2 * tp + 2)
                for ib in range(IB):
                    isl = slice(ib * NB, (ib + 1) * NB)
                    nc.tensor.matmul(
                        ps_s[jt, ib][:],
                        k8[:, tsl, jsl],
                        q8[:, tsl, isl],
                        start=(tp == 0), stop=(tp == TP - 1),
                        perf_mode=DR,
                    )
            s_exp(jt, 0)
            if jt >= 2:
                s_exp(jt - 2, 1)
            # den MMs: ib0 pairs as soon as both exps exist; ib1 two jts late
            if jt % 2 == 1:
                s = jt // 2
                ssl = slice(2 * s, 2 * s + 2)
                nc.tensor.matmul(
                    ps_den[0][:], ones8[:, :, 0:1], est8[:, ssl, 0:NB],
                    start=(s == 0), stop=(s == SP - 1),
                    perf_mode=DR, skip_group_check=True,
                )
                if s >= 1:
                    pssl = slice(2 * (s - 1), 2 * (s - 1) + 2)
                    nc.tensor.matmul(
                        ps_den[1][:], ones8[:, :, 0:1], est8[:, pssl, NB:HW],
                        start=(s == 1), stop=False,
                        perf_mode=DR, skip_group_check=True,
                    )
        s_exp(6, 1)
        rep_chain_act(0)
        s_exp(7, 1)
        nc.tensor.matmul(
            ps_den[1][:], ones8[:, :, 0:1], est8[:, 6:8, NB:HW],
            start=False, stop=True, perf_mode=DR, skip_group_check=True,
        )
        rep_chain_act(1)

        # ---- U = vT8 @ est8 (DoubleRow); u8 = psu * rep at the DVE drain --
        # rep matmuls slot in after the 2nd/6th U group so the PE never
        # stalls on the (ACT-backlogged) reciprocal chain; u8 drains are
        # deferred two groups so each is emitted AFTER its rep broadcast.
        u8 = big.tile([P, CT, HW], F8, tag="u8")
        pend = []

        def u_drain(ib, ct, psu):
            isl = slice(ib * NB, (ib + 1) * NB)
            nc.vector.tensor_tensor(
                out=u8[:, ct, isl], in0=psu[:], in1=rep[:, isl],
                op=mybir.AluOpType.mult,
            )

        for ib in range(IB):
            isl = slice(ib * NB, (ib + 1) * NB)
            for ct in range(CT):
                csl = slice(ct * P, (ct + 1) * P)
                psu = ps_pool.tile([P, NB], F32, tag="mmps", name="psu")
                for s in range(SP):
                    ssl = slice(2 * s, 2 * s + 2)
                    nc.tensor.matmul(
                        psu[:],
                        vT8[:, ssl, csl],
                        est8[:, ssl, isl],
                        start=(s == 0), stop=(s == SP - 1),
                        perf_mode=DR,
                    )
                if ib == 0 and ct == 1:
                    rep_mm(0)
                if ib == 1 and ct == 1:
                    rep_mm(1)
                pend.append((ib, ct, psu))
                while len(pend) > 2:
                    u_drain(*pend.pop(0))
        while pend:
            u_drain(*pend.pop(0))

        # ---- proj (fp8 DoubleRow) + residual on DVE -> staged output ----
        # drains land in a staging tile; each ot ships as ONE 512KB DMA on
        # its own ring as soon as both ib halves are drained (per-ring
        # transfers serialize, so one transfer per ring minimizes the tail).
        outr = out.rearrange("(t p) i -> p t i", p=P)
        ost = big.tile([P, CT, HW], F32, tag="ost")
        out_engs = [nc.sync, nc.gpsimd, nc.scalar, nc.sync]
        for ib in range(IB):
            isl = slice(ib * NB, (ib + 1) * NB)
            for ot in range(CT):
                osl = slice(ot * P, (ot + 1) * P)
                psp = ps_pool.tile([P, NB], F32, tag="mmps", name="psp")
                for tp in range(TP):
                    tsl = slice(2 * tp, 2 * tp + 2)
                    nc.tensor.matmul(
                        psp[:],
                        wp_sb[:, tsl, osl],
                        u8[:, tsl, isl],
                        start=(tp == 0), stop=False,
                        perf_mode=DR,
                    )
                # residual folded into the accumulation: psp += I.T @ x
                nc.tensor.matmul(
                    psp[:], id_sb[:], xsb[:, ot, isl],
                    start=False, stop=True, skip_group_check=True,
                )
                # drain = copy + bproj bias on ACT (idle in this phase)
                nc.scalar.activation(
                    out=ost[:, ot, isl], in_=psp[:], func=Identity,
                    bias=bp_sb[:, ot : ot + 1],
                )
                if ib == IB - 1:
                    out_engs[ot].dma_start(
                        out=outr[:, ot, :], in_=ost[:, ot, :]
                    )
    return nc


_NC = None


def _get_nc():
    global _NC
    if _NC is None:
        _NC = _build()
    return _NC


def _prep_inputs(x, gn_scale, gn_bias, wq, bq, wk, bk, wv, bv, wproj, bproj):
    import ml_dtypes

    f = np.float32
    f8 = ml_dtypes.float8_e4m3
    x = np.ascontiguousarray(x, dtype=f).reshape(B, C, HW)

    def t8(w):  # [o, c] -> [c, o] in fp8
        return np.asarray(w, dtype=f).T.astype(f8)

    def pt(v):  # [512] -> [128, 4] with v[t*128 + p] at [p, t]
        return np.ascontiguousarray(np.asarray(v, dtype=f).reshape(CT, P).T)

    pidx = np.arange(P)[:, None]
    tidx = np.arange(CT)[None, :]
    grp = 2 * tidx + pidx // GS  # [128, 4] group id per (p, t)
    gmat = np.zeros((P, CT, NG), f)
    hmat = np.zeros((P, CT, P), f)
    for t in range(CT):
        gmat[pidx[:, 0], t, grp[:, t]] = 1.0
        hmat[grp[:, t], t, pidx[:, 0]] = 1.0

    smallcat = np.concatenate(
        [
            pt(np.asarray(bq, dtype=f) * np.float32(SCALE)),
            pt(bk), pt(bproj), pt(gn_scale), pt(gn_bias),
            gmat.reshape(P, CT * NG), hmat.reshape(P, CT * P),
        ],
        axis=1,
    )
    w = np.stack([t8(wq), t8(wk), t8(wv), t8(wproj)])  # [j, c, o]
    w = w.reshape(4, CT, P, C).transpose(2, 0, 1, 3)  # [p, j, t, o]
    shared = {
        "wcat8": np.ascontiguousarray(w.reshape(P, 4 * CT * C)),
        "ident": np.ascontiguousarray(np.eye(P, dtype=f)),
        "bv": np.ascontiguousarray(np.asarray(bv, dtype=f)),
        "smallcat": np.ascontiguousarray(smallcat),
    }
    return [dict(shared, x=np.ascontiguousarray(x[b])) for b in range(B)]


def _run(inputs, **kw):
    nc = _get_nc()
    in_maps = _prep_inputs(**inputs)
    return run_bass_kernel_spmd(nc, in_maps, core_ids=list(range(B)), **kw)


def kernel(**inputs) -> np.ndarray:
    res = _run(inputs)
    out = np.stack([res.results[b]["out"] for b in range(B)])
    return out.reshape(B, C, HH, WW).astype(np.float32)
